# revision 31
# baseline (speedup 1.0000x reference)
"""Trainium2 Bass kernel for nn_DPP: batched masked-Gram logdet minus shared
normalizer logdet.

out[i] = logdet(G_sel_i) - logdet(G + I),  G = B^T B  (unit-norm columns)

Sharding (8 cores, one sample each):
  - Shared Gram G is SHARDED: core c computes square strips c and c+8 of G
    from fp8 B (DoubleRow matmuls), then two AllGathers (strips 0-7, 8-15)
    broadcast all strips; the collectives overlap with the masked-side work.
  - Masked term is COMPACT: the host gathers each sample's selected columns
    into Bsel [N, SB*128]; the device computes the compact Gram Bsel^T Bsel
    (equal to the selected submatrix of G) directly -- no mask vector ops,
    and the masked Cholesky shrinks from 16 panels to SB (~9).
  - logdet(G+I) is computed redundantly on every core (its Cholesky zips
    with the masked one to hide pivot-refinement latency).

Both factorizations use the baseline's matmul-only "refine" pivot scheme:
blocked left-looking U^T U Cholesky, 128-wide panels; each pivot block is
inverted approximately via a Neumann-type series with the logdet corrected
by tr F - tr F^2/2 + tr F^3/3.
"""

import numpy as np
import ml_dtypes

P = 128
N = 2048           # n (columns of B); also padded contraction dim (2000->2048)
NT = N // P        # 16 shared panels
NKT = 16           # contraction k-tiles
FT = 512           # free-dim tile for wide matmuls

_CACHE = {}


def _col_tiles(width_blocks, base_col, diag_first=False):
    tiles = []
    c = base_col
    end = base_col + width_blocks * P
    if diag_first:
        tiles.append((c, P))
        c += P
    while c < end:
        w = min(FT, end - c)
        tiles.append((c, w))
        c += w
    return tiles


def _build(SB):
    import concourse.bass as bass
    import concourse.bacc as bacc
    import concourse.mybir as mybir
    from concourse.bass import ds, ts
    from concourse.masks import (
        make_identity,
        make_upper_triangular,
        make_lower_triangular,
    )
    from concourse.tile import TileContext
    from contextlib import ExitStack
    from collections import deque

    f32 = mybir.dt.float32
    bf16 = mybir.dt.bfloat16
    f8 = mybir.dt.float8e4
    AF = mybir.ActivationFunctionType
    OP = mybir.AluOpType
    PSUM = bass.MemorySpace.PSUM
    AX = mybir.AxisListType.X
    DR = mybir.MatmulPerfMode.DoubleRow

    SP = SB * P
    NB = [SB, NT]          # panels per matrix: 0 = masked(compact), 1 = shared

    nc = bacc.Bacc()
    bb = nc.dram_tensor("bb", [N, N], f8, kind="ExternalInput")
    lhsg_d = nc.dram_tensor("lhsg", [N, 2 * P], f8, kind="ExternalInput")
    bsel_d = nc.dram_tensor("bsel", [N, SP], f8, kind="ExternalInput")
    vfix_d = nc.dram_tensor("vfix", [SP, 1], f32, kind="ExternalInput")
    out_d = nc.dram_tensor("out", [1, 1], f32, kind="ExternalOutput")

    with TileContext(nc) as tc, ExitStack() as stack:
        consts = stack.enter_context(tc.tile_pool(name="consts", bufs=1))
        I128 = consts.tile([P, P], f32, tag="i128")
        make_identity(nc, I128)
        I128b = consts.tile([P, P], bf16, tag="i128b")
        nc.vector.tensor_copy(I128b, I128)
        nI128b = consts.tile([P, P], bf16, tag="ni128b")
        nc.vector.tensor_scalar_mul(nI128b, I128, -1.0)
        STRIU = consts.tile([P, P], f32, tag="striu")
        make_upper_triangular(nc, STRIU, val=1.0, diag=False)
        STRIL = consts.tile([P, P], f32, tag="stril")
        make_lower_triangular(nc, STRIL, val=1.0, diag=False)
        vcol = consts.tile([P, SB], f32, tag="vcol")
        nc.sync.dma_start(vcol, vfix_d.rearrange("(t p) one -> p (t one)", p=P))
        acc = consts.tile([P, 2], f32, tag="acc")
        nc.vector.memset(acc, 0.0)
        dstore = consts.tile([P, 2, NT], f32, tag="dstore")
        nc.vector.memset(dstore.rearrange("p a b -> p (a b)"), 1.0)
        dfixm = consts.tile([P, SB, P], f32, tag="dfixm")
        for i in range(SB):
            nc.vector.tensor_scalar_mul(dfixm[:, i, :], I128, vcol[:, ds(i, 1)])

        # masked compact gram strips (read-only inputs to the masked chol)
        mgs = []
        for i in range(SB):
            mgs.append(consts.tile([P, (SB - i) * P], bf16, tag=f"mgs{i}",
                                   name=f"mgs{i}"))
        # fp8 U panels in a shifted-diagonal flat layout: panel j's absolute
        # column c sits at flat offset j*(W+P) + (c - j*P). Reading through a
        # row-length-W view places term j of any accumulation at [:, j, c]
        # with the SAME column c for every j, so DoubleRow pairs (j, j+1)
        # are plain [:, j:j+2, c] slices.
        WW = [SP, N]
        u8m = consts.tile([P, SB * (SP + P)], f8, tag="u8m", name="u8m")
        u8s = consts.tile([P, NT * (N + P)], f8, tag="u8s", name="u8s")
        u8 = [u8m, u8s]
        uview = [u8[0].rearrange("p (a b) -> p a b", b=SP),
                 u8[1].rearrange("p (a b) -> p a b", b=N)]
        wview = [u8[0].rearrange("p (a b) -> p a b", b=SP + P),
                 u8[1].rearrange("p (a b) -> p a b", b=N + P)]

        bpool = stack.enter_context(tc.tile_pool(name="bpool", bufs=1))
        gpsum = stack.enter_context(tc.tile_pool(name="gram_psum", bufs=2, space=PSUM))
        spool = stack.enter_context(tc.tile_pool(name="strip_pool", bufs=2))
        rpool = stack.enter_context(tc.tile_pool(name="ref_pool", bufs=2))
        vpool = stack.enter_context(tc.tile_pool(name="vec_pool", bufs=2))
        sstp = stack.enter_context(tc.tile_pool(name="sst_pool", bufs=3))
        apsum = stack.enter_context(tc.tile_pool(name="acc_psum", bufs=2, space=PSUM))
        wpsum = stack.enter_context(tc.tile_pool(name="work_psum", bufs=4, space=PSUM))
        fpsum = gpsum   # filler-class accum chains share the gram psum pool
        dram = stack.enter_context(tc.tile_pool(name="dram", bufs=1, space="DRAM"))

        lhs = bpool.tile([P, NKT, 2 * P], f8, tag="lhs")
        nc.sync.dma_start(lhs, lhsg_d.rearrange("(t p) w -> p t w", p=P))
        bt = bpool.tile([P, NKT, N], f8, tag="bt")
        bb_r = bb.rearrange("(t p) n -> p t n", p=P)
        for q in range(N // FT):
            nc.sync.dma_start(bt[:, :, ds(q * FT, FT)], bb_r[:, :, ds(q * FT, FT)])
        bs = bpool.tile([P, NKT, SP], f8, tag="bs")
        bsel_r = bsel_d.rearrange("(t p) s -> p t s", p=P)
        for q in range(SB):
            nc.sync.dma_start(bs[:, :, ds(q * P, P)], bsel_r[:, :, ds(q * P, P)])

        cin = [dram.tile([P, N], bf16, tag=f"cin{h}", name=f"cin{h}") for h in range(2)]
        cout = [dram.tile([8, P, N], bf16, tag=f"cout{h}", name=f"cout{h}")
                for h in range(2)]

        # ---- phase A: this core's two square G strips + AllGathers ----
        for h in range(2):
            stt = spool.tile([P, N], bf16, tag="stt", name=f"stt{h}")
            for ft in range(N // FT):
                pt = gpsum.tile([P, FT], f32, tag="gp", name="pt")
                for kt in range(0, NKT, 2):
                    nc.tensor.matmul(
                        pt,
                        lhs[:, kt:kt + 2, ds(h * P, P)],
                        bt[:, kt:kt + 2, ds(ft * FT, FT)],
                        start=(kt == 0),
                        stop=(kt == NKT - 2),
                        perf_mode=DR,
                    )
                nc.scalar.copy(stt[:, ds(ft * FT, FT)], pt)
            nc.gpsimd.dma_start(cin[h], stt)
            nc.gpsimd.collective_compute(
                "AllGather",
                mybir.AluOpType.bypass,
                replica_groups=[list(range(8))],
                ins=[cin[h].opt()],
                outs=[cout[h].opt()],
            )

        # ---- local copies of shared strips 0-1 (dodge the AllGather wait
        # ---- for the first two shared panels; every core computes them) --
        sst_loc = []
        for li in range(2):
            slt = consts.tile([P, (NT - li) * P], bf16, tag=f"sstloc{li}",
                              name=f"sstloc{li}")
            sst_loc.append(slt)

        def local_strip(li):
            for (c0, w) in _col_tiles(NT - li, li * P):
                pt = gpsum.tile([P, FT], f32, tag="gp", name="lpt")
                for kt in range(0, NKT, 2):
                    nc.tensor.matmul(
                        pt[:, :w],
                        bt[:, kt:kt + 2, ts(li, P)],
                        bt[:, kt:kt + 2, ds(c0, w)],
                        start=(kt == 0),
                        stop=(kt == NKT - 2),
                        perf_mode=DR,
                    )
                    if kt % 6 == 4:
                        yield
                nc.scalar.copy(sst_loc[li][:, ds(c0 - li * P, w)], pt[:, :w])
                yield

        # ---- masked compact gram strip generators (emitted as filler) ----
        def mgram_strip(i):
            for (c0, w) in _col_tiles(SB - i, i * P):
                pt = gpsum.tile([P, FT], f32, tag="gp", name="mgp")
                for kt in range(0, NKT, 2):
                    nc.tensor.matmul(
                        pt[:, :w],
                        bs[:, kt:kt + 2, ts(i, P)],
                        bs[:, kt:kt + 2, ds(c0, w)],
                        start=(kt == 0),
                        stop=(kt == NKT - 2),
                        perf_mode=DR,
                    )
                    if kt % 6 == 4:
                        yield
                nc.scalar.copy(mgs[i][:, ds(c0 - i * P, w)], pt[:, :w])
                yield

        mg_gens = [mgram_strip(i) for i in range(SB)]
        loc_gens = [local_strip(0), local_strip(1)]
        fillers = deque([loc_gens[0], loc_gens[1]] + mg_gens[:])

        def force_gen(g):
            for _ in g:
                pass
            if g in fillers:
                fillers.remove(g)

        def filler_step():
            while fillers:
                try:
                    next(fillers[0])
                    return
                except StopIteration:
                    fillers.popleft()

        # ---- panel machinery (shared by both matrices) ------------------
        # Pipelined: panel i+1's diagonal accumulation chain is emitted
        # while panel i's refine runs (all but the last term); right after
        # panel i's early TRSM tiles, one matmul finishes the diagonal and
        # the next refine starts. Off-diagonal tiles trail as fillers.

        def accum_chain(cx, tix, lo, hi, ap=None, stop=False):
            """Accumulate U_j^T U_j terms j in [lo, hi) for tile tix,
            DoubleRow-paired over the fp8 U panels."""
            m, i = cx["m"], cx["i"]
            c0, w = cx["tiles"][tix]
            uv = uview[m]
            if ap is None and hi > lo:
                ap = apsum.tile([P, FT], f32, tag="ap", name="ap")
            j = lo
            while j < hi:
                nj = 2 if j + 1 < hi else 1
                nc.tensor.matmul(
                    ap[:, :w],
                    uv[:, j:j + nj, ds(i * P, P)],
                    uv[:, j:j + nj, ds(c0, w)],
                    start=(j == 0),
                    stop=(stop and j + nj >= hi),
                    perf_mode=(DR if nj == 2 else None),
                )
                j += nj
            return ap

        def emit_prep_tile(cx, tix):
            m, i = cx["m"], cx["i"]
            c0, w = cx["tiles"][tix]
            gsl = cx["gsl"][:, ds(c0 - i * P, w)]
            ap = cx["aps"].get(tix)
            st = cx["strip"][:, ds(c0 - i * P, w)]
            if i > 0:
                nc.vector.tensor_sub(st, gsl, ap[:, :w])
            elif tix == 0:
                nc.vector.tensor_copy(st, gsl)
            if tix == 0:
                dfix = dfixm[:, i, :] if m == 0 else I128
                eng = nc.gpsimd if m == 0 else nc.vector
                eng.tensor_add(cx["sblk"], cx["strip"][:, ds(0, P)], dfix)
                eng.tensor_copy(cx["sb"], cx["sblk"])

        def make_cx(m, i):
            """Pre-start panel i: strip fetch + partial diag accum (j<i-1)."""
            wblk = NB[m] - i
            cx = {"m": m, "i": i, "aps": {},
                  "tiles": _col_tiles(wblk, i * P)}
            if m == 0:
                cx["gsl"] = mgs[i]
            elif i < 2:
                force_gen(loc_gens[i])
                cx["gsl"] = sst_loc[i]
            else:
                sst = sstp.tile([P, N], bf16, tag="sst", name=f"sst{i}")
                nc.sync.dma_start(
                    sst[:, :wblk * P],
                    cout[i // 8][i % 8, :, ds(i * P, wblk * P)],
                )
                cx["gsl"] = sst
            cx["sblk"] = rpool.tile([P, P], f32, tag="sblk", name="sblk")
            cx["strip"] = spool.tile([P, wblk * P], bf16, tag="strip", name="strip")
            cx["sb"] = cx["strip"][:, ds(0, P)]
            if i > 1:
                cx["aps"][0] = accum_chain(cx, 0, 0, i - 1)
            return cx

        def activate_panel(cx):
            """Finish diag accum (term j=i-1) and prep the diag block."""
            m, i = cx["m"], cx["i"]
            if m == 0:
                for j in range(i + 1):
                    force_gen(mg_gens[j])
            if i > 0:
                cx["aps"][0] = accum_chain(
                    cx, 0, max(0, i - 1), i, ap=cx["aps"].get(0), stop=True
                )
            emit_prep_tile(cx, 0)

        def make_prep_rest(cx):
            """Off-diagonal accum+prep tiles; emitted lazily as filler."""
            def rest():
                i, uv = cx["i"], uview[cx["m"]]
                for tix in range(1, len(cx["tiles"])):
                    if i > 0:
                        c0, w = cx["tiles"][tix]
                        ap = fpsum.tile([P, FT], f32, tag="gp", name="fap")
                        cx["aps"][tix] = ap
                        j = 0
                        while j < i:
                            nj = 2 if j + 1 < i else 1
                            nc.tensor.matmul(
                                ap[:, :w],
                                uv[:, j:j + nj, ds(i * P, P)],
                                uv[:, j:j + nj, ds(c0, w)],
                                start=(j == 0),
                                stop=(j + nj >= i),
                                perf_mode=(DR if nj == 2 else None),
                            )
                            j += nj
                            if j % 4 == 0:
                                yield
                        emit_prep_tile(cx, tix)
                    yield
            cx["rest"] = rest()
            fillers.appendleft(cx["rest"])

        def refine_gen(m, i, cx):
            sblk, sb = cx["sblk"], cx["sb"]
            dcol = dstore[:, m, ds(i, 1)]
            dummy = rpool.tile([P, P], f32, tag="dummy", name="dummy")
            nc.gpsimd.tensor_mul(dummy, sblk, I128)
            nc.vector.tensor_reduce(dcol, dummy, AX, OP.add)
            rinv = vpool.tile([P, 1], f32, tag="rinv", name="rinv")
            nc.vector.reciprocal(rinv, dcol)
            rcol = vpool.tile([P, 1], f32, tag="rcol", name="rcol")
            nc.scalar.sqrt(rcol, rinv)
            # c1 = diag(r) S diag(r) via two row-scales + a PE transpose
            rs = rpool.tile([P, P], bf16, tag="rs", name="rs")
            nc.scalar.mul(rs, sblk, rcol)
            yield
            rt_ps = wpsum.tile([P, FT * 2], bf16, tag="w", name="rt_ps")
            nc.tensor.transpose(rt_ps[:, :P], rs, I128b)
            c1 = rpool.tile([P, P], bf16, tag="c1", name="c1")
            nc.scalar.mul(c1, rt_ps[:, :P], rcol)
            yield
            x1 = rpool.tile([P, P], bf16, tag="x1", name="x1")
            nc.gpsimd.tensor_mul(x1, c1, STRIU)
            x1t = rpool.tile([P, P], bf16, tag="x1t", name="x1t")
            nc.gpsimd.tensor_mul(x1t, c1, STRIL)
            yield
            # x2_ps accumulates X1X1 - X1 + I entirely on PE
            x2_ps = wpsum.tile([P, FT], f32, tag="w", name="x2_ps")
            nc.tensor.matmul(x2_ps[:, :P], x1t, x1, start=True, stop=False)
            nc.tensor.matmul(x2_ps[:, :P], nI128b, x1, start=False, stop=False)
            nc.tensor.matmul(x2_ps[:, :P], I128b, I128b, start=False, stop=True)
            wfac = rpool.tile([P, P], bf16, tag="wfac", name="wfac")
            nc.scalar.mul(wfac, x2_ps[:, :P], rcol)
            yield
            wt_ps = wpsum.tile([P, FT * 2], bf16, tag="w", name="wt_ps")
            nc.tensor.transpose(wt_ps[:, :P], wfac, I128b)
            wt = rpool.tile([P, P], bf16, tag="wt", name="wt")
            nc.scalar.copy(wt, wt_ps[:, :P])
            yield
            sw_ps = wpsum.tile([P, FT], f32, tag="w", name="sw_ps")
            nc.tensor.matmul(sw_ps[:, :P], sb, wfac, start=True, stop=True)
            swt = rpool.tile([P, P], bf16, tag="swt", name="swt")
            nc.scalar.copy(swt, sw_ps[:, :P])
            yield
            # fpi_ps = W^T S W - I on PE; scalar engine derives F pieces
            fpi_ps = wpsum.tile([P, FT], f32, tag="w", name="fpi_ps")
            nc.tensor.matmul(fpi_ps[:, :P], wfac, swt, start=True, stop=False)
            nc.tensor.matmul(fpi_ps[:, :P], nI128b, I128b, start=False, stop=True)
            ff = rpool.tile([P, P], bf16, tag="ff", name="ff")
            nc.scalar.copy(ff, fpi_ps[:, :P])
            fs = rpool.tile([P, P], bf16, tag="fs", name="fs")
            nc.scalar.mul(fs, fpi_ps[:, :P], -0.5)
            yield
            f2_ps = wpsum.tile([P, FT], f32, tag="w", name="f2_ps")
            nc.tensor.matmul(f2_ps[:, :P], ff, ff, start=True, stop=True)
            f2s = rpool.tile([P, P], bf16, tag="f2s", name="f2s")
            nc.scalar.mul(f2s, f2_ps[:, :P], 0.375)
            yield
            # What = W + W(-F/2) + W(3F^2/8) accumulated on PE
            wh_ps = wpsum.tile([P, FT], f32, tag="w", name="wh_ps")
            nc.tensor.matmul(wh_ps[:, :P], wt, fs, start=True, stop=False)
            nc.tensor.matmul(wh_ps[:, :P], wt, f2s, start=False, stop=False)
            nc.tensor.matmul(wh_ps[:, :P], I128b, wfac, start=False, stop=True)
            what = rpool.tile([P, P], bf16, tag="what", name="what")
            nc.scalar.copy(what, wh_ps[:, :P])
            cx["what"] = what
            # trace series (off the critical path: only feeds `acc`)
            trf = vpool.tile([P, 1], f32, tag="trf", name="trf")
            dummy3 = rpool.tile([P, P], f32, tag="dummy3", name="dummy3")
            nc.gpsimd.tensor_mul(dummy3, ff, I128)
            nc.vector.tensor_reduce(trf, dummy3, AX, OP.add)
            trf2 = vpool.tile([P, 1], f32, tag="trf2", name="trf2")
            dummy4 = rpool.tile([P, P], f32, tag="dummy4", name="dummy4")
            nc.gpsimd.tensor_mul(dummy4, ff, ff)
            nc.vector.tensor_reduce(trf2, dummy4, AX, OP.add)
            trf3 = vpool.tile([P, 1], f32, tag="trf3", name="trf3")
            dummy5 = rpool.tile([P, P], f32, tag="dummy5", name="dummy5")
            nc.vector.tensor_mul(dummy5, f2_ps[:, :P], ff)
            nc.vector.tensor_reduce(trf3, dummy5, AX, OP.add)
            t1 = vpool.tile([P, 1], f32, tag="t1", name="t1")
            t2 = vpool.tile([P, 1], f32, tag="t2", name="t2")
            nc.vector.tensor_scalar(
                out=t2, in0=trf2, scalar1=-0.5, scalar2=None, op0=OP.mult
            )
            nc.vector.tensor_add(t1, trf, t2)
            nc.vector.tensor_scalar(
                out=t2, in0=trf3, scalar1=1.0 / 3.0, scalar2=None, op0=OP.mult
            )
            nc.vector.tensor_add(t1, t1, t2)
            nc.vector.tensor_add(acc[:, ds(m, 1)], acc[:, ds(m, 1)], t1)

        def trsm_tile(cx, tix):
            m, i = cx["m"], cx["i"]
            c0, w = cx["tiles"][tix]
            if i == 0 and tix > 0:
                rhs = cx["gsl"][:, ds(c0, w)]
            else:
                rhs = cx["strip"][:, ds(c0 - i * P, w)]
            tp = wpsum.tile([P, FT], f32, tag="w", name="tp")
            nc.tensor.matmul(tp[:, :w], cx["what"], rhs, start=True, stop=True)
            nc.scalar.copy(wview[m][:, i, ds(c0 - i * P, w)], tp[:, :w])

        # ---- phase C: software-pipelined interleaved panel schedule ----
        GATE = 2          # shared panels start after this many masked panels
        cur = {}          # matrix -> (refine gen, cx)
        pre = {}          # matrix -> pre-started next panel cx
        ready = {}        # matrix -> activated panel whose refine awaits pacing
        started = [0, 0]  # refines started per matrix

        def bootstrap(m):
            cx = make_cx(m, 0)
            activate_panel(cx)
            make_prep_rest(cx)
            cur[m] = (refine_gen(m, 0, cx), cx)
            started[m] += 1
            if NB[m] > 1:
                pre[m] = make_cx(m, 1)

        def advance(m, cx):
            """Refine of panel i done: early TRSM, hand off to panel i+1."""
            i = cx["i"]
            force_gen(cx["rest"])               # off-diag preps of panel i
            trsm_tile(cx, 0)                    # early: diag block ...
            if len(cx["tiles"]) > 1:
                trsm_tile(cx, 1)                # ... and first 512 tile
            nxt = pre.pop(m, None)
            if nxt is not None:
                activate_panel(nxt)             # final diag term + diag prep
            for tix in range(2, len(cx["tiles"])):
                trsm_tile(cx, tix)
            if nxt is not None:
                make_prep_rest(nxt)             # reads full ub_i: after trsm
                ready[m] = nxt
                if nxt["i"] + 1 < NB[m]:
                    pre[m] = make_cx(m, nxt["i"] + 1)

        def pace_ok(m):
            if m == 1 or not shared_on[0] or (1 not in cur and 1 not in ready):
                return True
            # stretch the remaining masked panels across the shared panels
            # so the shared tail keeps a zip partner
            return started[0] < GATE + 1 + (started[1] * (SB - GATE - 1)) // max(1, NT - 1)

        bootstrap(0)
        shared_on = [False]
        while cur or ready:
            for m in (0, 1):
                if m not in cur and m in ready and pace_ok(m):
                    cx = ready.pop(m)
                    cur[m] = (refine_gen(m, cx["i"], cx), cx)
                    started[m] += 1
                if m in cur:
                    g, cx = cur[m]
                    try:
                        next(g)
                    except StopIteration:
                        del cur[m]
                        advance(m, cx)
            filler_step()
            if not shared_on[0] and (started[0] >= GATE or (0 not in cur and 0 not in ready)):
                shared_on[0] = True
                bootstrap(1)
        while fillers:
            force_gen(fillers[0])

        # ---- final: batched Ln(d), partition-sum via matmul ----
        lnall = vpool.tile([P, 2, NT], f32, tag="lnall", name="lnall")
        nc.scalar.activation(
            lnall.rearrange("p a b -> p (a b)"),
            dstore.rearrange("p a b -> p (a b)"), AF.Ln,
        )
        ln0 = vpool.tile([P, 1], f32, tag="ln0", name="ln0")
        nc.vector.tensor_reduce(ln0, lnall[:, 0, :], AX, OP.add)
        ln1 = vpool.tile([P, 1], f32, tag="ln1", name="ln1")
        nc.vector.tensor_reduce(ln1, lnall[:, 1, :], AX, OP.add)
        accd = vpool.tile([P, 1], f32, tag="accd", name="accd")
        nc.vector.tensor_sub(accd, acc[:, 0:1], acc[:, 1:2])
        nc.vector.tensor_add(accd, accd, ln0)
        nc.vector.tensor_sub(accd, accd, ln1)
        ones = vpool.tile([P, 1], f32, tag="ones", name="ones")
        nc.vector.memset(ones, 1.0)
        r_ps = wpsum.tile([P, FT], f32, tag="w", name="r_ps")
        nc.tensor.matmul(r_ps[:1, :1], accd, ones, start=True, stop=True)
        res = vpool.tile([1, 1], f32, tag="res", name="res")
        nc.vector.tensor_copy(res, r_ps[:1, :1])
        nc.sync.dma_start(out_d[:, :], res)

    nc.finalize()
    return nc


def prep_in_maps(x, B, SB):
    """Host-side sharding: per-core fp8 inputs."""
    f8 = ml_dtypes.float8_e4m3
    k, n = B.shape
    SPp = SB * P
    bpad8 = np.zeros((N, N), dtype=f8)
    bpad8[:k, :] = B.astype(f8)
    in_maps = []
    for c in range(8):
        sel = np.flatnonzero(x[c] == 1)
        s = len(sel)
        bsel = np.zeros((N, SPp), dtype=f8)
        bsel[:k, :s] = B[:, sel].astype(f8)
        vfix = np.zeros((SPp, 1), dtype=np.float32)
        vfix[s:] = 1.0
        lhsg = np.concatenate(
            [bpad8[:, c * P:(c + 1) * P], bpad8[:, (c + 8) * P:(c + 9) * P]],
            axis=1,
        )
        in_maps.append({
            "bb": bpad8, "lhsg": np.ascontiguousarray(lhsg),
            "bsel": bsel, "vfix": vfix,
        })
    return in_maps


def kernel(x, B):
    """Full inputs -> full output. x: [8, 2048] int32, B: [2000, 2048] f32."""
    from concourse.bass_utils import run_bass_kernel_spmd

    bs_, n = x.shape
    assert n == N and bs_ == 8
    s = (np.asarray(x) == 1).sum(axis=1)
    SB = max(2, -(-int(s.max()) // P))
    if SB not in _CACHE:
        _CACHE[SB] = _build(SB)
    nc = _CACHE[SB]
    in_maps = prep_in_maps(np.asarray(x), np.asarray(B, dtype=np.float32), SB)
    res = run_bass_kernel_spmd(nc, in_maps, core_ids=list(range(8)))
    return np.array([r["out"][0, 0] for r in res.results], dtype=np.float32)


# revision 33
# speedup vs baseline: 1.2129x; 1.2129x over previous
"""Trainium2 Bass kernel for nn_DPP: batched masked-Gram logdet minus shared
normalizer logdet.

out[i] = logdet(G_sel_i) - logdet(G + I),  G = B^T B  (unit-norm columns)

Sharding (8 cores, one sample each):
  - Shared Gram G is SHARDED: core c computes square strips c and c+8 of G
    from fp8 B (DoubleRow matmuls), then two AllGathers (strips 0-7, 8-15)
    broadcast all strips; the collectives overlap with the masked-side work.
  - Masked term is COMPACT: the host gathers each sample's selected columns
    into Bsel [N, SB*128]; the device computes the compact Gram Bsel^T Bsel
    (equal to the selected submatrix of G) directly -- no mask vector ops,
    and the masked Cholesky shrinks from 16 panels to SB (~9).
  - logdet(G+I) is computed redundantly on every core (its Cholesky zips
    with the masked one to hide pivot-refinement latency).

Both factorizations use the baseline's matmul-only "refine" pivot scheme:
blocked left-looking U^T U Cholesky, 128-wide panels; each pivot block is
inverted approximately via a Neumann-type series with the logdet corrected
by tr F - tr F^2/2 + tr F^3/3.
"""

import numpy as np
import ml_dtypes

P = 128
N = 2048           # n (columns of B); also padded contraction dim (2000->2048)
NT = N // P        # 16 shared panels
NKT = 16           # contraction k-tiles
FT = 512           # free-dim tile for wide matmuls

_CACHE = {}


def _col_tiles(width_blocks, base_col, diag_first=False):
    tiles = []
    c = base_col
    end = base_col + width_blocks * P
    if diag_first:
        tiles.append((c, P))
        c += P
    while c < end:
        w = min(FT, end - c)
        tiles.append((c, w))
        c += w
    return tiles


def _build(SB):
    import concourse.bass as bass
    import concourse.bacc as bacc
    import concourse.mybir as mybir
    from concourse.bass import ds, ts
    from concourse.masks import (
        make_identity,
        make_upper_triangular,
        make_lower_triangular,
    )
    from concourse.tile import TileContext
    from contextlib import ExitStack
    from collections import deque

    f32 = mybir.dt.float32
    bf16 = mybir.dt.bfloat16
    f8 = mybir.dt.float8e4
    AF = mybir.ActivationFunctionType
    OP = mybir.AluOpType
    PSUM = bass.MemorySpace.PSUM
    AX = mybir.AxisListType.X
    DR = mybir.MatmulPerfMode.DoubleRow

    SP = SB * P
    NB = [SB, NT]          # panels per matrix: 0 = masked(compact), 1 = shared

    nc = bacc.Bacc()
    bb = nc.dram_tensor("bb", [N, N], f8, kind="ExternalInput")
    lhsg_d = nc.dram_tensor("lhsg", [N, 2 * P], f8, kind="ExternalInput")
    bsel_d = nc.dram_tensor("bsel", [N, SP], f8, kind="ExternalInput")
    vfix_d = nc.dram_tensor("vfix", [SP, 1], f32, kind="ExternalInput")
    out_d = nc.dram_tensor("out", [1, 1], f32, kind="ExternalOutput")

    with TileContext(nc) as tc, ExitStack() as stack:
        consts = stack.enter_context(tc.tile_pool(name="consts", bufs=1))
        I128 = consts.tile([P, P], f32, tag="i128")
        make_identity(nc, I128)
        I128b = consts.tile([P, P], bf16, tag="i128b")
        nc.vector.tensor_copy(I128b, I128)
        nI128b = consts.tile([P, P], bf16, tag="ni128b")
        nc.vector.tensor_scalar_mul(nI128b, I128, -1.0)
        STRIU = consts.tile([P, P], f32, tag="striu")
        make_upper_triangular(nc, STRIU, val=1.0, diag=False)
        STRIL = consts.tile([P, P], f32, tag="stril")
        make_lower_triangular(nc, STRIL, val=1.0, diag=False)
        vcol = consts.tile([P, SB], f32, tag="vcol")
        nc.sync.dma_start(vcol, vfix_d.rearrange("(t p) one -> p (t one)", p=P))
        acc = consts.tile([P, 2], f32, tag="acc")
        nc.vector.memset(acc, 0.0)
        dstore = consts.tile([P, 2, NT], f32, tag="dstore")
        nc.vector.memset(dstore.rearrange("p a b -> p (a b)"), 1.0)
        dfixm = consts.tile([P, SB, P], f32, tag="dfixm")
        for i in range(SB):
            nc.vector.tensor_scalar_mul(dfixm[:, i, :], I128, vcol[:, ds(i, 1)])

        # masked compact gram strips (read-only inputs to the masked chol)
        mgs = []
        for i in range(SB):
            mgs.append(consts.tile([P, (SB - i) * P], bf16, tag=f"mgs{i}",
                                   name=f"mgs{i}"))
        # fp8 U panels in a shifted-diagonal flat layout: panel j's absolute
        # column c sits at flat offset j*(W+P) + (c - j*P). Reading through a
        # row-length-W view places term j of any accumulation at [:, j, c]
        # with the SAME column c for every j, so DoubleRow pairs (j, j+1)
        # are plain [:, j:j+2, c] slices.
        WW = [SP, N]
        u8m = consts.tile([P, SB * (SP + P)], f8, tag="u8m", name="u8m")
        u8s = consts.tile([P, NT * (N + P)], f8, tag="u8s", name="u8s")
        u8 = [u8m, u8s]
        uview = [u8[0].rearrange("p (a b) -> p a b", b=SP),
                 u8[1].rearrange("p (a b) -> p a b", b=N)]
        wview = [u8[0].rearrange("p (a b) -> p a b", b=SP + P),
                 u8[1].rearrange("p (a b) -> p a b", b=N + P)]

        bpool = stack.enter_context(tc.tile_pool(name="bpool", bufs=1))
        gpsum = stack.enter_context(tc.tile_pool(name="gram_psum", bufs=2, space=PSUM))
        spool = stack.enter_context(tc.tile_pool(name="strip_pool", bufs=2))
        rpool = stack.enter_context(tc.tile_pool(name="ref_pool", bufs=2))
        vpool = stack.enter_context(tc.tile_pool(name="vec_pool", bufs=2))
        sstp = stack.enter_context(tc.tile_pool(name="sst_pool", bufs=3))
        apsum = stack.enter_context(tc.tile_pool(name="acc_psum", bufs=2, space=PSUM))
        wpsum = stack.enter_context(tc.tile_pool(name="work_psum", bufs=4, space=PSUM))
        fpsum = gpsum   # filler-class accum chains share the gram psum pool
        dram = stack.enter_context(tc.tile_pool(name="dram", bufs=1, space="DRAM"))

        lhs = bpool.tile([P, NKT, 2 * P], f8, tag="lhs")
        nc.sync.dma_start(lhs, lhsg_d.rearrange("(t p) w -> p t w", p=P))
        bt = bpool.tile([P, NKT, N], f8, tag="bt")
        bb_r = bb.rearrange("(t p) n -> p t n", p=P)
        for q in range(N // FT):
            nc.sync.dma_start(bt[:, :, ds(q * FT, FT)], bb_r[:, :, ds(q * FT, FT)])
        bs = bpool.tile([P, NKT, SP], f8, tag="bs")
        bsel_r = bsel_d.rearrange("(t p) s -> p t s", p=P)
        for q in range(SB):
            nc.sync.dma_start(bs[:, :, ds(q * P, P)], bsel_r[:, :, ds(q * P, P)])

        cin = [dram.tile([P, N], bf16, tag=f"cin{h}", name=f"cin{h}") for h in range(2)]
        cout = [dram.tile([8, P, N], bf16, tag=f"cout{h}", name=f"cout{h}")
                for h in range(2)]

        # ---- phase A: this core's two square G strips + AllGathers ----
        for h in range(2):
            stt = spool.tile([P, N], bf16, tag="stt", name=f"stt{h}")
            for ft in range(N // FT):
                pt = gpsum.tile([P, FT], f32, tag="gp", name="pt")
                for kt in range(0, NKT, 2):
                    nc.tensor.matmul(
                        pt,
                        lhs[:, kt:kt + 2, ds(h * P, P)],
                        bt[:, kt:kt + 2, ds(ft * FT, FT)],
                        start=(kt == 0),
                        stop=(kt == NKT - 2),
                        perf_mode=DR,
                    )
                nc.scalar.copy(stt[:, ds(ft * FT, FT)], pt)
            nc.gpsimd.dma_start(cin[h], stt)
            nc.gpsimd.collective_compute(
                "AllGather",
                mybir.AluOpType.bypass,
                replica_groups=[list(range(8))],
                ins=[cin[h].opt()],
                outs=[cout[h].opt()],
            )

        # ---- local copies of shared strips 0-1 (dodge the AllGather wait
        # ---- for the first two shared panels; every core computes them) --
        sst_loc = []
        for li in range(2):
            slt = consts.tile([P, (NT - li) * P], bf16, tag=f"sstloc{li}",
                              name=f"sstloc{li}")
            sst_loc.append(slt)

        def local_strip(li):
            for (c0, w) in _col_tiles(NT - li, li * P):
                pt = gpsum.tile([P, FT], f32, tag="gp", name="lpt")
                for kt in range(0, NKT, 2):
                    nc.tensor.matmul(
                        pt[:, :w],
                        bt[:, kt:kt + 2, ts(li, P)],
                        bt[:, kt:kt + 2, ds(c0, w)],
                        start=(kt == 0),
                        stop=(kt == NKT - 2),
                        perf_mode=DR,
                    )
                    if kt % 6 == 4:
                        yield
                nc.scalar.copy(sst_loc[li][:, ds(c0 - li * P, w)], pt[:, :w])
                yield

        # ---- masked compact gram strip generators (emitted as filler) ----
        def mgram_strip(i):
            for (c0, w) in _col_tiles(SB - i, i * P):
                pt = gpsum.tile([P, FT], f32, tag="gp", name="mgp")
                for kt in range(0, NKT, 2):
                    nc.tensor.matmul(
                        pt[:, :w],
                        bs[:, kt:kt + 2, ts(i, P)],
                        bs[:, kt:kt + 2, ds(c0, w)],
                        start=(kt == 0),
                        stop=(kt == NKT - 2),
                        perf_mode=DR,
                    )
                    if kt % 6 == 4:
                        yield
                nc.scalar.copy(mgs[i][:, ds(c0 - i * P, w)], pt[:, :w])
                yield

        mg_gens = [mgram_strip(i) for i in range(SB)]
        loc_gens = [local_strip(0), local_strip(1)]
        fillers = deque([loc_gens[0], loc_gens[1]] + mg_gens[:])

        def force_gen(g):
            for _ in g:
                pass
            if g in fillers:
                fillers.remove(g)

        def filler_step():
            while fillers:
                try:
                    next(fillers[0])
                    return
                except StopIteration:
                    fillers.popleft()

        # ---- panel machinery (shared by both matrices) ------------------
        # Pipelined: panel i+1's diagonal accumulation chain is emitted
        # while panel i's refine runs (all but the last term); right after
        # panel i's early TRSM tiles, one matmul finishes the diagonal and
        # the next refine starts. Off-diagonal tiles trail as fillers.

        def accum_chain(cx, tix, lo, hi, ap=None, stop=False):
            """Accumulate U_j^T U_j terms j in [lo, hi) for tile tix,
            DoubleRow-paired over the fp8 U panels."""
            m, i = cx["m"], cx["i"]
            c0, w = cx["tiles"][tix]
            uv = uview[m]
            if ap is None and hi > lo:
                ap = apsum.tile([P, FT], f32, tag="ap", name="ap")
            j = lo
            while j < hi:
                nj = 2 if j + 1 < hi else 1
                nc.tensor.matmul(
                    ap[:, :w],
                    uv[:, j:j + nj, ds(i * P, P)],
                    uv[:, j:j + nj, ds(c0, w)],
                    start=(j == 0),
                    stop=(stop and j + nj >= hi),
                    perf_mode=(DR if nj == 2 else None),
                )
                j += nj
            return ap

        def emit_prep_tile(cx, tix):
            m, i = cx["m"], cx["i"]
            c0, w = cx["tiles"][tix]
            gsl = cx["gsl"][:, ds(c0 - i * P, w)]
            ap = cx["aps"].get(tix)
            st = cx["strip"][:, ds(c0 - i * P, w)]
            if i > 0:
                nc.vector.tensor_sub(st, gsl, ap[:, :w])
            elif tix == 0:
                nc.vector.tensor_copy(st, gsl)
            if tix == 0:
                dfix = dfixm[:, i, :] if m == 0 else I128
                if m == 0:
                    nc.gpsimd.tensor_add(cx["sblk"], cx["strip"][:, ds(0, P)], dfix)
                    nc.gpsimd.tensor_copy(cx["sb"], cx["sblk"])
                else:
                    nc.vector.tensor_add(cx["sblk"], cx["strip"][:, ds(0, P)], dfix)
                    nc.scalar.copy(cx["sb"], cx["sblk"])

        def make_cx(m, i):
            """Pre-start panel i: strip fetch + partial diag accum (j<i-1)."""
            wblk = NB[m] - i
            cx = {"m": m, "i": i, "aps": {},
                  "tiles": _col_tiles(wblk, i * P)}
            if m == 0:
                cx["gsl"] = mgs[i]
            elif i < 2:
                force_gen(loc_gens[i])
                cx["gsl"] = sst_loc[i]
            else:
                sst = sstp.tile([P, N], bf16, tag="sst", name=f"sst{i}")
                nc.sync.dma_start(
                    sst[:, :wblk * P],
                    cout[i // 8][i % 8, :, ds(i * P, wblk * P)],
                )
                cx["gsl"] = sst
            cx["sblk"] = rpool.tile([P, P], f32, tag="sblk", name="sblk")
            cx["strip"] = spool.tile([P, wblk * P], bf16, tag="strip", name="strip")
            cx["sb"] = cx["strip"][:, ds(0, P)]
            if i > 1:
                cx["aps"][0] = accum_chain(cx, 0, 0, i - 1)
            return cx

        def activate_panel(cx):
            """Finish diag accum (term j=i-1) and prep the diag block."""
            m, i = cx["m"], cx["i"]
            if m == 0:
                for j in range(i + 1):
                    force_gen(mg_gens[j])
            if i > 0:
                cx["aps"][0] = accum_chain(
                    cx, 0, max(0, i - 1), i, ap=cx["aps"].get(0), stop=True
                )
            emit_prep_tile(cx, 0)

        def make_prep_rest(cx):
            """Off-diagonal accum+prep tiles; emitted lazily as filler."""
            def rest():
                i, uv = cx["i"], uview[cx["m"]]
                for tix in range(1, len(cx["tiles"])):
                    if i > 0:
                        c0, w = cx["tiles"][tix]
                        ap = fpsum.tile([P, FT], f32, tag="gp", name="fap")
                        cx["aps"][tix] = ap
                        j = 0
                        while j < i:
                            nj = 2 if j + 1 < i else 1
                            nc.tensor.matmul(
                                ap[:, :w],
                                uv[:, j:j + nj, ds(i * P, P)],
                                uv[:, j:j + nj, ds(c0, w)],
                                start=(j == 0),
                                stop=(j + nj >= i),
                                perf_mode=(DR if nj == 2 else None),
                            )
                            j += nj
                            if j % 4 == 0:
                                yield
                        emit_prep_tile(cx, tix)
                    yield
            cx["rest"] = rest()
            fillers.appendleft(cx["rest"])

        def refine_gen(m, i, cx):
            # Engine split so the two factorizations' chains don't
            # head-of-line block each other: masked -> Vector, shared ->
            # Scalar (activation copy/scale handles PSUM reads and the
            # per-partition rcol scaling).
            if m == 0:
                cp = nc.vector.tensor_copy
                scl = nc.vector.tensor_scalar_mul
            else:
                cp = nc.scalar.copy
                scl = nc.scalar.mul
            sblk, sb = cx["sblk"], cx["sb"]
            dcol = dstore[:, m, ds(i, 1)]
            dummy = rpool.tile([P, P], f32, tag="dummy", name="dummy")
            nc.gpsimd.tensor_mul(dummy, sblk, I128)
            nc.vector.tensor_reduce(dcol, dummy, AX, OP.add)
            rinv = vpool.tile([P, 1], f32, tag="rinv", name="rinv")
            nc.vector.reciprocal(rinv, dcol)
            rcol = vpool.tile([P, 1], f32, tag="rcol", name="rcol")
            nc.scalar.sqrt(rcol, rinv)
            # c1 = diag(r) S diag(r) via two row-scales + a PE transpose
            rs = rpool.tile([P, P], bf16, tag="rs", name="rs")
            scl(rs, sblk, rcol)
            yield
            rt_ps = wpsum.tile([P, FT * 2], bf16, tag="w", name="rt_ps")
            nc.tensor.transpose(rt_ps[:, :P], rs, I128b)
            c1 = rpool.tile([P, P], bf16, tag="c1", name="c1")
            scl(c1, rt_ps[:, :P], rcol)
            yield
            x1 = rpool.tile([P, P], bf16, tag="x1", name="x1")
            nc.gpsimd.tensor_mul(x1, c1, STRIU)
            x1t = rpool.tile([P, P], bf16, tag="x1t", name="x1t")
            nc.gpsimd.tensor_mul(x1t, c1, STRIL)
            yield
            # x2_ps accumulates X1X1 - X1 + I entirely on PE
            x2_ps = wpsum.tile([P, FT], f32, tag="w", name="x2_ps")
            nc.tensor.matmul(x2_ps[:, :P], x1t, x1, start=True, stop=False)
            nc.tensor.matmul(x2_ps[:, :P], nI128b, x1, start=False, stop=False)
            nc.tensor.matmul(x2_ps[:, :P], I128b, I128b, start=False, stop=True)
            wfac = rpool.tile([P, P], bf16, tag="wfac", name="wfac")
            scl(wfac, x2_ps[:, :P], rcol)
            yield
            wt_ps = wpsum.tile([P, FT * 2], bf16, tag="w", name="wt_ps")
            nc.tensor.transpose(wt_ps[:, :P], wfac, I128b)
            wt = rpool.tile([P, P], bf16, tag="wt", name="wt")
            cp(wt, wt_ps[:, :P])
            yield
            sw_ps = wpsum.tile([P, FT], f32, tag="w", name="sw_ps")
            nc.tensor.matmul(sw_ps[:, :P], sb, wfac, start=True, stop=True)
            swt = rpool.tile([P, P], bf16, tag="swt", name="swt")
            cp(swt, sw_ps[:, :P])
            yield
            # fpi_ps = W^T S W - I on PE
            fpi_ps = wpsum.tile([P, FT], f32, tag="w", name="fpi_ps")
            nc.tensor.matmul(fpi_ps[:, :P], wfac, swt, start=True, stop=False)
            nc.tensor.matmul(fpi_ps[:, :P], nI128b, I128b, start=False, stop=True)
            ff = rpool.tile([P, P], bf16, tag="ff", name="ff")
            cp(ff, fpi_ps[:, :P])
            fs = rpool.tile([P, P], bf16, tag="fs", name="fs")
            scl(fs, fpi_ps[:, :P], -0.5)
            yield
            f2_ps = wpsum.tile([P, FT], f32, tag="w", name="f2_ps")
            nc.tensor.matmul(f2_ps[:, :P], ff, ff, start=True, stop=True)
            f2s = rpool.tile([P, P], bf16, tag="f2s", name="f2s")
            scl(f2s, f2_ps[:, :P], 0.375)
            yield
            # What = W + W(-F/2) + W(3F^2/8) accumulated on PE
            wh_ps = wpsum.tile([P, FT], f32, tag="w", name="wh_ps")
            nc.tensor.matmul(wh_ps[:, :P], wt, fs, start=True, stop=False)
            nc.tensor.matmul(wh_ps[:, :P], wt, f2s, start=False, stop=False)
            nc.tensor.matmul(wh_ps[:, :P], I128b, wfac, start=False, stop=True)
            what = rpool.tile([P, P], bf16, tag="what", name="what")
            cp(what, wh_ps[:, :P])
            cx["what"] = what
            # trace series (off the critical path: only feeds `acc`)
            trf = vpool.tile([P, 1], f32, tag="trf", name="trf")
            dummy3 = rpool.tile([P, P], f32, tag="dummy3", name="dummy3")
            nc.gpsimd.tensor_mul(dummy3, ff, I128)
            nc.vector.tensor_reduce(trf, dummy3, AX, OP.add)
            trf2 = vpool.tile([P, 1], f32, tag="trf2", name="trf2")
            dummy4 = rpool.tile([P, P], f32, tag="dummy4", name="dummy4")
            nc.gpsimd.tensor_mul(dummy4, ff, ff)
            nc.vector.tensor_reduce(trf2, dummy4, AX, OP.add)
            trf3 = vpool.tile([P, 1], f32, tag="trf3", name="trf3")
            dummy5 = rpool.tile([P, P], f32, tag="dummy5", name="dummy5")
            nc.vector.tensor_mul(dummy5, f2_ps[:, :P], ff)
            nc.vector.tensor_reduce(trf3, dummy5, AX, OP.add)
            t1 = vpool.tile([P, 1], f32, tag="t1", name="t1")
            t2 = vpool.tile([P, 1], f32, tag="t2", name="t2")
            nc.vector.tensor_scalar(
                out=t2, in0=trf2, scalar1=-0.5, scalar2=None, op0=OP.mult
            )
            nc.vector.tensor_add(t1, trf, t2)
            nc.vector.tensor_scalar(
                out=t2, in0=trf3, scalar1=1.0 / 3.0, scalar2=None, op0=OP.mult
            )
            nc.vector.tensor_add(t1, t1, t2)
            nc.vector.tensor_add(acc[:, ds(m, 1)], acc[:, ds(m, 1)], t1)

        def trsm_tile(cx, tix):
            m, i = cx["m"], cx["i"]
            c0, w = cx["tiles"][tix]
            if i == 0 and tix > 0:
                rhs = cx["gsl"][:, ds(c0, w)]
            else:
                rhs = cx["strip"][:, ds(c0 - i * P, w)]
            tp = wpsum.tile([P, FT], f32, tag="w", name="tp")
            nc.tensor.matmul(tp[:, :w], cx["what"], rhs, start=True, stop=True)
            nc.scalar.copy(wview[m][:, i, ds(c0 - i * P, w)], tp[:, :w])

        # ---- phase C: software-pipelined interleaved panel schedule ----
        GATE = 2          # shared panels start after this many masked panels
        cur = {}          # matrix -> (refine gen, cx)
        pre = {}          # matrix -> pre-started next panel cx
        ready = {}        # matrix -> activated panel whose refine awaits pacing
        started = [0, 0]  # refines started per matrix

        def bootstrap(m):
            cx = make_cx(m, 0)
            activate_panel(cx)
            make_prep_rest(cx)
            cur[m] = (refine_gen(m, 0, cx), cx)
            started[m] += 1
            if NB[m] > 1:
                pre[m] = make_cx(m, 1)

        def advance(m, cx):
            """Refine of panel i done: early TRSM, hand off to panel i+1."""
            i = cx["i"]
            force_gen(cx["rest"])               # off-diag preps of panel i
            trsm_tile(cx, 0)                    # early: diag block ...
            if len(cx["tiles"]) > 1:
                trsm_tile(cx, 1)                # ... and first 512 tile
            nxt = pre.pop(m, None)
            if nxt is not None:
                activate_panel(nxt)             # final diag term + diag prep
            for tix in range(2, len(cx["tiles"])):
                trsm_tile(cx, tix)
            if nxt is not None:
                make_prep_rest(nxt)             # reads full ub_i: after trsm
                ready[m] = nxt
                if nxt["i"] + 1 < NB[m]:
                    pre[m] = make_cx(m, nxt["i"] + 1)

        def pace_ok(m):
            if m == 1 or not shared_on[0] or (1 not in cur and 1 not in ready):
                return True
            # stretch the remaining masked panels across the shared panels
            # so the shared tail keeps a zip partner
            return started[0] < GATE + 1 + (started[1] * (SB - GATE - 1)) // max(1, NT - 1)

        bootstrap(0)
        shared_on = [False]
        while cur or ready:
            for m in (0, 1):
                if m not in cur and m in ready and pace_ok(m):
                    cx = ready.pop(m)
                    cur[m] = (refine_gen(m, cx["i"], cx), cx)
                    started[m] += 1
                if m in cur:
                    g, cx = cur[m]
                    try:
                        next(g)
                    except StopIteration:
                        del cur[m]
                        advance(m, cx)
            filler_step()
            if not shared_on[0] and (started[0] >= GATE or (0 not in cur and 0 not in ready)):
                shared_on[0] = True
                bootstrap(1)
        while fillers:
            force_gen(fillers[0])

        # ---- final: batched Ln(d), partition-sum via matmul ----
        lnall = vpool.tile([P, 2, NT], f32, tag="lnall", name="lnall")
        nc.scalar.activation(
            lnall.rearrange("p a b -> p (a b)"),
            dstore.rearrange("p a b -> p (a b)"), AF.Ln,
        )
        ln0 = vpool.tile([P, 1], f32, tag="ln0", name="ln0")
        nc.vector.tensor_reduce(ln0, lnall[:, 0, :], AX, OP.add)
        ln1 = vpool.tile([P, 1], f32, tag="ln1", name="ln1")
        nc.vector.tensor_reduce(ln1, lnall[:, 1, :], AX, OP.add)
        accd = vpool.tile([P, 1], f32, tag="accd", name="accd")
        nc.vector.tensor_sub(accd, acc[:, 0:1], acc[:, 1:2])
        nc.vector.tensor_add(accd, accd, ln0)
        nc.vector.tensor_sub(accd, accd, ln1)
        ones = vpool.tile([P, 1], f32, tag="ones", name="ones")
        nc.vector.memset(ones, 1.0)
        r_ps = wpsum.tile([P, FT], f32, tag="w", name="r_ps")
        nc.tensor.matmul(r_ps[:1, :1], accd, ones, start=True, stop=True)
        res = vpool.tile([1, 1], f32, tag="res", name="res")
        nc.vector.tensor_copy(res, r_ps[:1, :1])
        nc.sync.dma_start(out_d[:, :], res)

    nc.finalize()
    return nc


def prep_in_maps(x, B, SB):
    """Host-side sharding: per-core fp8 inputs."""
    f8 = ml_dtypes.float8_e4m3
    k, n = B.shape
    SPp = SB * P
    bpad8 = np.zeros((N, N), dtype=f8)
    bpad8[:k, :] = B.astype(f8)
    in_maps = []
    for c in range(8):
        sel = np.flatnonzero(x[c] == 1)
        s = len(sel)
        bsel = np.zeros((N, SPp), dtype=f8)
        bsel[:k, :s] = B[:, sel].astype(f8)
        vfix = np.zeros((SPp, 1), dtype=np.float32)
        vfix[s:] = 1.0
        lhsg = np.concatenate(
            [bpad8[:, c * P:(c + 1) * P], bpad8[:, (c + 8) * P:(c + 9) * P]],
            axis=1,
        )
        in_maps.append({
            "bb": bpad8, "lhsg": np.ascontiguousarray(lhsg),
            "bsel": bsel, "vfix": vfix,
        })
    return in_maps


def kernel(x, B):
    """Full inputs -> full output. x: [8, 2048] int32, B: [2000, 2048] f32."""
    from concourse.bass_utils import run_bass_kernel_spmd

    bs_, n = x.shape
    assert n == N and bs_ == 8
    s = (np.asarray(x) == 1).sum(axis=1)
    SB = max(2, -(-int(s.max()) // P))
    if SB not in _CACHE:
        _CACHE[SB] = _build(SB)
    nc = _CACHE[SB]
    in_maps = prep_in_maps(np.asarray(x), np.asarray(B, dtype=np.float32), SB)
    res = run_bass_kernel_spmd(nc, in_maps, core_ids=list(range(8)))
    return np.array([r["out"][0, 0] for r in res.results], dtype=np.float32)
